# revision 2
# baseline (speedup 1.0000x reference)
"""TRN2 Bass kernel for nn_CompiledBlock_45148696216108 (moe_routing).

Reference computation:
    xp  = x[permute]
    xn  = LayerNorm(xp; gamma, beta, eps=1e-5)
    yp  = xn @ W.T + b
    out = (xp + yp)[argsort(permute)]

The block (LayerNorm + Linear + residual) is purely row-wise, so the
gather by `permute` and the scatter by its inverse cancel exactly:
    out = x + LN(x) @ W.T + b
No token movement (and no cross-core all-to-all) is needed. Tokens are
sharded contiguously across the 8 cores; the tiny weights are folded on
the host and replicated:
    A[h, o] = gamma[h] * W[o, h]          (pre-transposed, gamma folded)
    s[o]    = sum_h A[h, o]
    c[o]    = sum_h beta[h] * W[o, h] + b[o]

LayerNorm is affine per token, so it commutes with the matmul:
    out = x + q * (x @ A) + r * s + c,   q = rsqrt(var + eps), r = -mu * q
This lets the PE consume x directly - no on-device transpose of the
normalized activations is needed, because the host stages x a second
time in a transposed, PE-native tiling (pure layout/dtype prep):
    xtc[tile, hp, kh, tt] = x[tile*128 + tt, kh*128 + hp]  (bf16)
Each [128, 128] chunk of that layout IS the matmul stationary operand.

Per-core device pipeline (tokens_per_core = 8192, HIDDEN = 2048):
  - DMA x tile (128 tokens, 2048, fp32; residual + stats) and the
    matching xtc tile (bf16, transposed tiling)
  - DVE bn_stats/bn_aggr -> mean/var; ACT sqrt; DVE reciprocal -> q;
    DVE r = -mu*q; ACT rs = r*s; GPSIMD x += c; DVE x += rs
  - PE: 64 bf16 matmuls, k-outer so each stationary chunk is reused for
    4 consecutive instructions (keeps LDWEIGHTS off the critical path):
    psum_j[tt, o512] += xtc[kh].T @ A[kh][:, j*512:(j+1)*512]
  - DVE per bank: out_j = psum_j * q + (x + c + r*s)_j  (one
    scalar_tensor_tensor with per-partition scalar q); DMA out
8 PSUM banks rotate across tiles (4 banks x 2 buffers) so the next
tile's accumulation never waits on this tile's drain.
"""

import numpy as np
from contextlib import ExitStack

import ml_dtypes

from concourse import bacc, tile, mybir
from concourse.bass_utils import run_bass_kernel_spmd

N_TOK = 65536
HIDDEN = 2048
N_CORES = 8
P = 128
EPS = 1e-5
F32 = mybir.dt.float32
BF16 = mybir.dt.bfloat16
AF = mybir.ActivationFunctionType
ALU = mybir.AluOpType

NB = 512               # matmul moving free dim (one PSUM bank of fp32)
KC = HIDDEN // P       # 16 contraction chunks
OC = HIDDEN // NB      # 4 output column chunks


def build(tokens_per_core: int = N_TOK // N_CORES, num_devices: int = N_CORES):
    T = tokens_per_core
    NT = T // P            # token tiles

    nc = bacc.Bacc(
        "TRN2", target_bir_lowering=False, debug=False, num_devices=num_devices
    )
    x_d = nc.dram_tensor("x", [T, HIDDEN], F32, kind="ExternalInput").ap()
    xtc_d = nc.dram_tensor("xtc", [NT, P, HIDDEN], BF16, kind="ExternalInput").ap()
    a_d = nc.dram_tensor("A", [KC, P, HIDDEN], BF16, kind="ExternalInput").ap()
    c_d = nc.dram_tensor("c", [P, HIDDEN], F32, kind="ExternalInput").ap()
    s_d = nc.dram_tensor("s", [P, HIDDEN], F32, kind="ExternalInput").ap()
    out_d = nc.dram_tensor("out", [T, HIDDEN], F32, kind="ExternalOutput").ap()

    with tile.TileContext(nc) as tc, ExitStack() as ctx:
        const = ctx.enter_context(tc.tile_pool(name="const", bufs=1))
        apool = ctx.enter_context(tc.tile_pool(name="apool", bufs=1))
        xpool = ctx.enter_context(tc.tile_pool(name="xpool", bufs=3))
        xtcpool = ctx.enter_context(tc.tile_pool(name="xtcpool", bufs=3))
        outpool = ctx.enter_context(tc.tile_pool(name="outpool", bufs=3))
        stpool = ctx.enter_context(tc.tile_pool(name="stats", bufs=3))
        rspool = ctx.enter_context(tc.tile_pool(name="rspool", bufs=2))
        psy_pool = ctx.enter_context(tc.tile_pool(name="psy", bufs=2, space="PSUM"))

        # Prefetch the first x/xtc tiles BEFORE the 8 MB weight DMA so the
        # stats prologue runs during the weight load.
        xts, xtts = {}, {}
        for i in range(2):
            xts[i] = xpool.tile([P, HIDDEN], F32, tag="xt", name=f"xt_pre{i}")
            nc.sync.dma_start(xts[i][:], x_d[i * P : (i + 1) * P, :])
            xtts[i] = xtcpool.tile([P, HIDDEN], BF16, tag="xtt", name=f"xtt_pre{i}")
            nc.sync.dma_start(xtts[i][:], xtc_d[i])
        c_sb = const.tile([P, HIDDEN], F32)
        nc.sync.dma_start(c_sb[:], c_d[:])
        s_sb = const.tile([P, HIDDEN], F32)
        nc.sync.dma_start(s_sb[:], s_d[:])

        # Resident weights: 16 separately-tracked chunks of (128 h, 2048 o)
        # so matmuls on chunk k only wait for chunk k's DMA.
        a_sb = []
        for k in range(KC):
            ak = apool.tile([P, HIDDEN], BF16, tag=f"a{k}")
            nc.sync.dma_start(ak[:], a_d[k])
            a_sb.append(ak)
        eps_sb = const.tile([P, 1], F32)
        nc.gpsimd.memset(eps_sb[:], EPS)

        def prep(i):
            """Stats + residual pre-bias for tile i -> per-token scale q.

            Leaves xts[i] holding x + c + r*s (the stt addend) and returns
            the q tile.
            """
            xt = xts[i]
            stats = stpool.tile([P, 4, 6], F32, tag="stats")
            xr = xt[:].rearrange("p (a b) -> p a b", b=512)
            for a in range(4):
                nc.vector.bn_stats(stats[:, a, :], xr[:, a, :])
            mv = stpool.tile([P, 2], F32, tag="mv")
            nc.vector.bn_aggr(mv[:], stats[:])
            sig = stpool.tile([P, 1], F32, tag="sig")
            nc.scalar.activation(sig[:], mv[:, 1:2], AF.Sqrt, bias=eps_sb[:])
            q = stpool.tile([P, 1], F32, tag="q")
            nc.vector.reciprocal(q[:], sig[:])
            r = stpool.tile([P, 1], F32, tag="r")
            nc.vector.scalar_tensor_tensor(
                r[:], mv[:, 0:1], -1.0, q[:], ALU.mult, ALU.mult
            )
            rs = rspool.tile([P, HIDDEN], F32, tag="rs")
            nc.scalar.activation(rs[:], s_sb[:], AF.Identity, scale=r[:])
            # WAR on the bn_stats reads above: fold the constant bias and the
            # per-token -mu*q*s term into the residual in place.
            nc.gpsimd.tensor_add(xt[:], xt[:], c_sb[:])
            nc.vector.tensor_add(xt[:], xt[:], rs[:])
            return q

        qs = {}
        qs[0] = prep(0)

        for t in range(NT):
            xt = xts.pop(t)
            xtt = xtts.pop(t)
            q = qs.pop(t)

            # Prefetch + prep next tile while this tile's matmuls run
            if t + 2 < NT:
                xts[t + 2] = xpool.tile([P, HIDDEN], F32, tag="xt", name=f"xt_{t + 2}")
                nc.sync.dma_start(
                    xts[t + 2][:], x_d[(t + 2) * P : (t + 3) * P, :]
                )
                xtts[t + 2] = xtcpool.tile(
                    [P, HIDDEN], BF16, tag="xtt", name=f"xtt_{t + 2}"
                )
                nc.sync.dma_start(xtts[t + 2][:], xtc_d[t + 2])
            if t + 1 < NT:
                qs[t + 1] = prep(t + 1)

            # Matmuls: k-outer so the stationary operand (a 128x128 chunk of
            # the transposed x tile) is reused by 4 consecutive matmuls, one
            # per PSUM bank; all 4 banks accumulate across the kh sweep.
            psys = [
                psy_pool.tile([P, NB], F32, tag=f"psy{j}", name=f"psy_{t}_{j}")
                for j in range(OC)
            ]
            for k in range(KC):
                xk = xtt[:, k * P : (k + 1) * P]
                for j in range(OC):
                    nc.tensor.matmul(
                        psys[j][:],
                        xk,
                        a_sb[k][:, j * NB : (j + 1) * NB],
                        start=(k == 0),
                        stop=(k == KC - 1),
                    )

            ot = outpool.tile([P, HIDDEN], F32, tag="ot")
            for j in range(OC):
                sl = slice(j * NB, (j + 1) * NB)
                nc.vector.scalar_tensor_tensor(
                    ot[:, sl], psys[j][:], q[:], xt[:, sl], ALU.mult, ALU.add
                )
            nc.sync.dma_start(out_d[t * P : (t + 1) * P, :], ot[:])

    nc.compile()
    return nc


_built = None


def _get_built():
    global _built
    if _built is None:
        _built = build()
    return _built


def _prep_inputs(x, permute, gamma, beta, W, b):
    x = np.asarray(x, dtype=np.float32)
    gamma = np.asarray(gamma, dtype=np.float64)
    beta = np.asarray(beta, dtype=np.float64)
    W = np.asarray(W, dtype=np.float64)
    b = np.asarray(b, dtype=np.float64)
    A = W.T * gamma[:, None]                      # (H, O), gamma folded
    A_bf = A.astype(np.float32).astype(ml_dtypes.bfloat16)
    s = A_bf.astype(np.float64).sum(axis=0)       # matches the bf16 matmul
    A_bf = np.ascontiguousarray(A_bf.reshape(KC, P, HIDDEN))
    c = np.ascontiguousarray(
        np.broadcast_to((W @ beta + b).reshape(1, HIDDEN), (P, HIDDEN))
    ).astype(np.float32)
    s = np.ascontiguousarray(
        np.broadcast_to(s.reshape(1, HIDDEN), (P, HIDDEN))
    ).astype(np.float32)
    T = N_TOK // N_CORES
    NT = T // P
    in_maps = []
    for i in range(N_CORES):
        xs = x[i * T : (i + 1) * T]
        # Transposed, PE-native tiling: xtc[tile, hp, kh, tt] =
        # xs[tile*128 + tt, kh*128 + hp]; chunk kh of a tile is the matmul
        # stationary operand [K=hp, M=tt].
        xtc = np.ascontiguousarray(
            xs.reshape(NT, P, KC, P).transpose(0, 3, 2, 1)
        ).astype(ml_dtypes.bfloat16).reshape(NT, P, HIDDEN)
        in_maps.append({"x": xs, "xtc": xtc, "A": A_bf, "c": c, "s": s})
    return in_maps


def kernel(x, permute, gamma, beta, W, b):
    nc = _get_built()
    in_maps = _prep_inputs(x, permute, gamma, beta, W, b)
    res = run_bass_kernel_spmd(nc, in_maps, list(range(N_CORES))).results
    return np.concatenate([r["out"] for r in res], axis=0)


if __name__ == "__main__":
    rng = np.random.default_rng(0)
    x = rng.standard_normal((N_TOK, HIDDEN), dtype=np.float32)
    permute = rng.permutation(N_TOK).astype(np.int64)
    gamma = np.ones(HIDDEN, np.float32)
    beta = np.zeros(HIDDEN, np.float32)
    W = (rng.standard_normal((HIDDEN, HIDDEN), dtype=np.float32) / np.sqrt(HIDDEN))
    b = rng.standard_normal(HIDDEN, dtype=np.float32) * 0.01
    out = kernel(x=x, permute=permute, gamma=gamma, beta=beta, W=W, b=b)
    print(out.shape, out.dtype)


# revision 4
# speedup vs baseline: 1.2829x; 1.2829x over previous
"""TRN2 Bass kernel for nn_CompiledBlock_45148696216108 (moe_routing).

Reference computation:
    xp  = x[permute]
    xn  = LayerNorm(xp; gamma, beta, eps=1e-5)
    yp  = xn @ W.T + b
    out = (xp + yp)[argsort(permute)]

The block (LayerNorm + Linear + residual) is purely row-wise, so the
gather by `permute` and the scatter by its inverse cancel exactly:
    out = x + LN(x) @ W.T + b
No token movement (and no cross-core all-to-all) is needed. Tokens are
sharded contiguously across the 8 cores; the tiny weights are folded on
the host and replicated:
    A[h, o] = gamma[h] * W[o, h]          (pre-transposed, gamma folded)
    s[o]    = sum_h A[h, o]
    c[o]    = sum_h beta[h] * W[o, h] + b[o]

LayerNorm is affine per token, so it commutes with the matmul:
    out = x + q * (x @ A) + r * s + c,   q = rsqrt(var + eps), r = -mu * q
This lets the PE consume x directly - no on-device transpose of the
normalized activations is needed, because the host stages x a second
time in a transposed, PE-native tiling (pure layout/dtype prep):
    xtc[tile, hp, kh, tt] = x[tile*128 + tt, kh*128 + hp]  (bf16)
Each [128, 128] chunk of that layout IS the matmul stationary operand.

Per-core device pipeline (tokens_per_core = 8192, HIDDEN = 2048):
  - DMA x tile (128 tokens, 2048, fp32; residual + stats) and the
    matching xtc tile (bf16, transposed tiling)
  - DVE bn_stats/bn_aggr -> mean/var; ACT sqrt; DVE reciprocal -> q;
    DVE r = -mu*q; ACT rs = r*s; GPSIMD x += c; DVE x += rs
  - PE: 64 bf16 matmuls, k-outer so each stationary chunk is reused for
    4 consecutive instructions (keeps LDWEIGHTS off the critical path):
    psum_j[tt, o512] += xtc[kh].T @ A[kh][:, j*512:(j+1)*512]
  - DVE per bank: out_j = psum_j * q + (x + c + r*s)_j  (one
    scalar_tensor_tensor with per-partition scalar q); DMA out
8 PSUM banks rotate across tiles (4 banks x 2 buffers) so the next
tile's accumulation never waits on this tile's drain.
"""

import numpy as np
from contextlib import ExitStack

import ml_dtypes

from concourse import bacc, tile, mybir
from concourse.bass_utils import run_bass_kernel_spmd

N_TOK = 65536
HIDDEN = 2048
N_CORES = 8
P = 128
EPS = 1e-5
F32 = mybir.dt.float32
BF16 = mybir.dt.bfloat16
AF = mybir.ActivationFunctionType
ALU = mybir.AluOpType

NB = 512               # matmul moving free dim (one PSUM bank of fp32)
KC = HIDDEN // P       # 16 contraction chunks
OC = HIDDEN // NB      # 4 output column chunks


def build(tokens_per_core: int = N_TOK // N_CORES, num_devices: int = N_CORES):
    T = tokens_per_core
    NT = T // P            # token tiles

    nc = bacc.Bacc(
        "TRN2", target_bir_lowering=False, debug=False, num_devices=num_devices
    )
    x_d = nc.dram_tensor("x", [T, HIDDEN], F32, kind="ExternalInput").ap()
    xtc_d = nc.dram_tensor("xtc", [NT, P, HIDDEN], BF16, kind="ExternalInput").ap()
    a_d = nc.dram_tensor("A", [KC, P, HIDDEN], BF16, kind="ExternalInput").ap()
    c_d = nc.dram_tensor("c", [P, HIDDEN], F32, kind="ExternalInput").ap()
    s_d = nc.dram_tensor("s", [P, HIDDEN], F32, kind="ExternalInput").ap()
    out_d = nc.dram_tensor("out", [T, HIDDEN], F32, kind="ExternalOutput").ap()

    with tile.TileContext(nc) as tc, ExitStack() as ctx:
        const = ctx.enter_context(tc.tile_pool(name="const", bufs=1))
        apool = ctx.enter_context(tc.tile_pool(name="apool", bufs=1))
        xpool = ctx.enter_context(tc.tile_pool(name="xpool", bufs=3))
        xtcpool = ctx.enter_context(tc.tile_pool(name="xtcpool", bufs=3))
        outpool = ctx.enter_context(tc.tile_pool(name="outpool", bufs=3))
        stpool = ctx.enter_context(tc.tile_pool(name="stats", bufs=3))
        rspool = ctx.enter_context(tc.tile_pool(name="rspool", bufs=2))
        psy_pool = ctx.enter_context(tc.tile_pool(name="psy", bufs=2, space="PSUM"))

        # Prefetch the first x/xtc tiles BEFORE the 8 MB weight DMA so the
        # stats prologue runs during the weight load.
        xts, xtts = {}, {}
        for i in range(2):
            xts[i] = xpool.tile([P, HIDDEN], F32, tag="xt", name=f"xt_pre{i}")
            nc.sync.dma_start(xts[i][:], x_d[i * P : (i + 1) * P, :])
            xtts[i] = xtcpool.tile([P, HIDDEN], BF16, tag="xtt", name=f"xtt_pre{i}")
            nc.sync.dma_start(xtts[i][:], xtc_d[i])
        c_sb = const.tile([P, HIDDEN], F32)
        nc.sync.dma_start(c_sb[:], c_d[:])
        s_sb = const.tile([P, HIDDEN], F32)
        nc.sync.dma_start(s_sb[:], s_d[:])

        # Resident weights: 16 separately-tracked chunks of (128 h, 2048 o)
        # so matmuls on chunk k only wait for chunk k's DMA.
        a_sb = []
        for k in range(KC):
            ak = apool.tile([P, HIDDEN], BF16, tag=f"a{k}")
            nc.sync.dma_start(ak[:], a_d[k])
            a_sb.append(ak)
        eps_sb = const.tile([P, 1], F32)
        nc.gpsimd.memset(eps_sb[:], EPS)

        def prep(i):
            """Stats + residual pre-bias for tile i -> per-token scale q.

            Leaves xts[i] holding x + c + r*s (the stt addend) and returns
            the q tile.
            """
            xt = xts[i]
            stats = stpool.tile([P, 4, 6], F32, tag="stats")
            xr = xt[:].rearrange("p (a b) -> p a b", b=512)
            for a in range(4):
                nc.vector.bn_stats(stats[:, a, :], xr[:, a, :])
            mv = stpool.tile([P, 2], F32, tag="mv")
            nc.vector.bn_aggr(mv[:], stats[:])
            sig = stpool.tile([P, 1], F32, tag="sig")
            nc.scalar.activation(sig[:], mv[:, 1:2], AF.Sqrt, bias=eps_sb[:])
            q = stpool.tile([P, 1], F32, tag="q")
            nc.vector.reciprocal(q[:], sig[:])
            # p = mu*q on the scalar engine; s_sb holds -s so rs = -mu*q*s.
            p = stpool.tile([P, 1], F32, tag="p")
            nc.scalar.activation(p[:], mv[:, 0:1], AF.Identity, scale=q[:])
            rs = rspool.tile([P, HIDDEN], F32, tag="rs")
            nc.scalar.activation(rs[:], s_sb[:], AF.Identity, scale=p[:])
            # WAR on the bn_stats reads above: fold the constant bias and the
            # per-token -mu*q*s term into the residual in place.
            nc.gpsimd.tensor_add(xt[:], xt[:], c_sb[:])
            nc.gpsimd.tensor_add(xt[:], xt[:], rs[:])
            return q

        qs = {}
        qs[0] = prep(0)

        for t in range(NT):
            xt = xts.pop(t)
            xtt = xtts.pop(t)
            q = qs.pop(t)

            # Prefetch + prep next tile while this tile's matmuls run
            if t + 2 < NT:
                xts[t + 2] = xpool.tile([P, HIDDEN], F32, tag="xt", name=f"xt_{t + 2}")
                nc.sync.dma_start(
                    xts[t + 2][:], x_d[(t + 2) * P : (t + 3) * P, :]
                )
                xtts[t + 2] = xtcpool.tile(
                    [P, HIDDEN], BF16, tag="xtt", name=f"xtt_{t + 2}"
                )
                nc.sync.dma_start(xtts[t + 2][:], xtc_d[t + 2])
            if t + 1 < NT:
                qs[t + 1] = prep(t + 1)

            # Matmuls: k-outer so the stationary operand (a 128x128 chunk of
            # the transposed x tile) is reused by 4 consecutive matmuls, one
            # per PSUM bank; all 4 banks accumulate across the kh sweep.
            psys = [
                psy_pool.tile([P, NB], F32, tag=f"psy{j}", name=f"psy_{t}_{j}")
                for j in range(OC)
            ]
            for k in range(KC):
                xk = xtt[:, k * P : (k + 1) * P]
                for j in range(OC):
                    nc.tensor.matmul(
                        psys[j][:],
                        xk,
                        a_sb[k][:, j * NB : (j + 1) * NB],
                        start=(k == 0),
                        stop=(k == KC - 1),
                    )

            ot = outpool.tile([P, HIDDEN], F32, tag="ot")
            for j in range(OC):
                sl = slice(j * NB, (j + 1) * NB)
                nc.vector.scalar_tensor_tensor(
                    ot[:, sl], psys[j][:], q[:], xt[:, sl], ALU.mult, ALU.add
                )
            nc.sync.dma_start(out_d[t * P : (t + 1) * P, :], ot[:])

    nc.compile()
    return nc


_built = None


def _get_built():
    global _built
    if _built is None:
        _built = build()
    return _built


def _prep_inputs(x, permute, gamma, beta, W, b):
    x = np.asarray(x, dtype=np.float32)
    gamma = np.asarray(gamma, dtype=np.float64)
    beta = np.asarray(beta, dtype=np.float64)
    W = np.asarray(W, dtype=np.float64)
    b = np.asarray(b, dtype=np.float64)
    A = W.T * gamma[:, None]                      # (H, O), gamma folded
    A_bf = A.astype(np.float32).astype(ml_dtypes.bfloat16)
    s = -A_bf.astype(np.float64).sum(axis=0)      # negated; matches bf16 matmul
    A_bf = np.ascontiguousarray(A_bf.reshape(KC, P, HIDDEN))
    c = np.ascontiguousarray(
        np.broadcast_to((W @ beta + b).reshape(1, HIDDEN), (P, HIDDEN))
    ).astype(np.float32)
    s = np.ascontiguousarray(
        np.broadcast_to(s.reshape(1, HIDDEN), (P, HIDDEN))
    ).astype(np.float32)
    T = N_TOK // N_CORES
    NT = T // P
    in_maps = []
    for i in range(N_CORES):
        xs = x[i * T : (i + 1) * T]
        # Transposed, PE-native tiling: xtc[tile, hp, kh, tt] =
        # xs[tile*128 + tt, kh*128 + hp]; chunk kh of a tile is the matmul
        # stationary operand [K=hp, M=tt].
        xtc = np.ascontiguousarray(
            xs.reshape(NT, P, KC, P).transpose(0, 3, 2, 1)
        ).astype(ml_dtypes.bfloat16).reshape(NT, P, HIDDEN)
        in_maps.append({"x": xs, "xtc": xtc, "A": A_bf, "c": c, "s": s})
    return in_maps


def kernel(x, permute, gamma, beta, W, b):
    nc = _get_built()
    in_maps = _prep_inputs(x, permute, gamma, beta, W, b)
    res = run_bass_kernel_spmd(nc, in_maps, list(range(N_CORES))).results
    return np.concatenate([r["out"] for r in res], axis=0)


if __name__ == "__main__":
    rng = np.random.default_rng(0)
    x = rng.standard_normal((N_TOK, HIDDEN), dtype=np.float32)
    permute = rng.permutation(N_TOK).astype(np.int64)
    gamma = np.ones(HIDDEN, np.float32)
    beta = np.zeros(HIDDEN, np.float32)
    W = (rng.standard_normal((HIDDEN, HIDDEN), dtype=np.float32) / np.sqrt(HIDDEN))
    b = rng.standard_normal(HIDDEN, dtype=np.float32) * 0.01
    out = kernel(x=x, permute=permute, gamma=gamma, beta=beta, W=W, b=b)
    print(out.shape, out.dtype)
